# revision 20
# baseline (speedup 1.0000x reference)
"""CGCoupler Trainium2 Bass kernel: fp16 inputs, legal DVE/Pool/ACT split.

out[n, ro[k]] += x1[n, r1[k]] * x2[n, r2[k]] * cg[k]  for all k, rows n.

The CG index tables decompose into 147 contiguous-slice FMAs per row
(out[:, o:o+d] += c * x1[:, a:a+d] * x2[:, b:b+d], d in {32,64}).

Engine legality on real TRN2 (walrus ISA checks; CoreSim is looser):
TensorScalar/ScalarTensorTensor run ONLY on DVE; the Pool engine supports
TensorTensor/memset/copy/DMA; ACT supports activation (copy with scale).
ISA structs carry at most ONE embedded sync wait, and 4D-AP TensorTensor
carries none, so every cross-engine handoff goes through a wait-capable
absorber instruction on the consuming engine.

Structure:
 - x1/x2 are fp16 (converted on host).  X1 loads issue from SP, X2 from ACT
   (parallel HWDGE queues).  All products (fp16 TT, 0.52 ns/elem) run on DVE
   into a shared P buffer.
 - The output space splits at boundary B.  The DVE band accumulates with
   TS/STT as usual.  The Pool band instead: DVE/ACT compute per-term scaled
   products Q = c*P (DVE tensor_scalar at 0.26 ns/elem fp16, ACT activation
   copy-with-scale at 0.833; whole chunks assigned to one producer), Pool
   memsets its O band and accumulates with 3D TT-adds (0.833 ns/elem,
   coefficient-free so runs merge across ranges by layer).  A strided Pool
   copy per chunk absorbs the producer wait (robust to scheduler
   reordering); each chunk's store issues from ACT after a strided ACT
   absorber, so stores carry only their DMA-ring wait.
 - Rows ride the 128 SBUF partitions, T=8 row-tiles fold per instruction,
   2 groups per core; O is single-buffered (fits SBUF), X/P double-buffered.

Data-parallel across 8 NeuronCores: each core processes 2048 rows.
"""
import numpy as np

N_CORES = 8
P_DIM = 128
T_FOLD = 8

_BUILD_CACHE = {}


# ----------------------------------------------------------------------------
# Planning
# ----------------------------------------------------------------------------

def _extract_sliceops(cg, r1, r2, ro):
    M = len(cg)
    ops = []
    k = 0
    while k < M:
        j = k + 1
        while (j < M and r1[j] == r1[j-1] + 1 and r2[j] == r2[j-1] + 1
               and ro[j] == ro[j-1] + 1 and cg[j] == cg[k]):
            j += 1
        ops.append((int(r1[k]), int(r2[k]), int(ro[k]), j - k, float(cg[k])))
        k = j
    return ops


_RATES = {
    'DVE':  dict(TT=0.5208, STT=1.0417, TS=0.2604, ovh=105.0),
    'Pool': dict(ADD=0.8333, MEMSET=0.8333, ovh=40.0),
    'ACT':  dict(TS=0.8333, ovh=60.0),
}


def _merge_products(pairs, slot):
    instrs = []
    i = 0
    while i < len(pairs):
        a0, b0, d0 = pairs[i]
        s0 = slot[pairs[i]]
        j = i + 1
        da = db = ds = None
        while j < len(pairs):
            a1, b1, d1 = pairs[j]
            if d1 != d0:
                break
            nda = a1 - pairs[j-1][0]
            ndb = b1 - pairs[j-1][1]
            nds = slot[pairs[j]] - slot[pairs[j-1]]
            if da is None:
                da, db, ds = nda, ndb, nds
            elif (nda, ndb, nds) != (da, db, ds):
                break
            j += 1
        n = j - i
        if n == 1:
            da = db = ds = 0
        instrs.append(dict(pslot=s0, a=a0, b=b0, d=d0, da=da, db=db, ds=ds, n=n))
        i = j
    return instrs


def _merge_runs(qs, need_c):
    """Merge ops with contiguous (o, pslot) runs of equal d (and equal c when
    need_c).  qs must carry o/pslot/c/d/kind."""
    instrs = []
    i = 0
    while i < len(qs):
        q0 = qs[i]
        j = i + 1
        while j < len(qs):
            q1, qp = qs[j], qs[j-1]
            if q1['kind'] != q0['kind'] or q1['d'] != q0['d']:
                break
            if need_c and q1['c'] != q0['c']:
                break
            if q1['o'] - qp['o'] != q0['d'] or q1['pslot'] - qp['pslot'] != q0['d']:
                break
            j += 1
        n = j - i
        instrs.append(dict(kind=q0['kind'], o=q0['o'], pslot=q0['pslot'],
                           c=q0['c'], d=q0['d'], n=n))
        i = j
    return instrs


def _plan_dve_chunk(ops_c, lo, hi, slot):
    covered = np.zeros(hi - lo, bool)
    qs = []
    raw = sorted(ops_c, key=lambda t: (-t[3], t[4], t[2]))
    for (a, b, o, d, c) in raw:
        rng = slice(o - lo, o - lo + d)
        kind = 'TS' if not covered[rng].any() else 'STT'
        assert kind == 'TS' or covered[rng].all()
        covered[rng] = True
        qs.append(dict(o=o, pslot=slot[(a, b, d)], c=c, d=d, kind=kind))
    assert covered.all()
    qs = sorted(qs, key=lambda q: (q['kind'] != 'TS', -q['d'], q['c'],
                                   q['o'], q['pslot']))
    return _merge_runs(qs, need_c=True)


def _plan_pool_chunk(ops_c, lo, hi, slot, qbase):
    """Pool-band chunk: per-op Q slots laid out layer-major so TT-adds merge
    across ranges without a coefficient constraint."""
    layer_of = {}
    items = []
    for (a, b, o, d, c) in sorted(ops_c, key=lambda t: (t[2], -t[3])):
        l = layer_of.get(o, 0)
        layer_of[o] = l + 1
        items.append(dict(o=o, pslot=slot[(a, b, d)], c=c, d=d, layer=l))
    items.sort(key=lambda q: (q['layer'], q['o']))
    qoff = qbase
    for it in items:
        it['qslot'] = qoff
        qoff += it['d']
    # Q instructions: qslot contiguous by construction; merge needs same c
    qgen = [dict(kind='Q', o=it['qslot'], pslot=it['pslot'], c=it['c'],
                 d=it['d']) for it in items]
    q_instrs = _merge_runs(qgen, need_c=True)
    # TT-adds: out[o] += Q[qslot]; merge runs regardless of c
    agen = [dict(kind='ADD', o=it['o'], pslot=it['qslot'], c=0.0, d=it['d'])
            for it in items]
    add_instrs = _merge_runs(agen, need_c=False)
    return q_instrs, add_instrs, qoff


def _chunk_bounds(band_ops, out_lo, out_hi, bounds, n_chunks, taper=0.8):
    valid = [b for b in bounds if out_lo < b < out_hi]
    work = np.zeros(out_hi - out_lo)
    for (a, b, o, d, c) in band_ops:
        work[o - out_lo:o - out_lo + d] += 1
    total = work.sum()
    weights = np.array([taper ** i for i in range(n_chunks)])
    weights = weights / weights.sum()
    chunks = []
    cur_lo = out_lo
    remaining = n_chunks
    for ci in range(n_chunks - 1):
        target = total * weights[ci]
        best_b, best_gap = None, None
        for b in valid:
            if b - cur_lo < 128 or out_hi - b < 128 * (remaining - 1):
                continue
            w = work[cur_lo - out_lo:b - out_lo].sum()
            gap = abs(w - target)
            if best_gap is None or gap < best_gap:
                best_b, best_gap = b, gap
        if best_b is None:
            break
        chunks.append((cur_lo, best_b))
        cur_lo = best_b
        remaining -= 1
    chunks.append((cur_lo, out_hi))
    return chunks


def _build_plan(cg, r1, r2, ro, out_dim, T=T_FOLD, force=None):
    ops = _extract_sliceops(cg, r1, r2, ro)

    cand = set(range(32, out_dim, 32))
    for (_, _, o, d, _) in ops:
        for x in range(o + 32, o + d, 32):
            cand.discard(x)
    bounds = sorted(cand)

    def build_cfg(B):
        # DVE band = [0, B), Pool band = [B, out_dim)
        lo_ops = [t for t in ops if t[2] + t[3] <= B]
        hi_ops = [t for t in ops if t[2] >= B]
        assert len(lo_ops) + len(hi_ops) == len(ops)
        # shared P layout: pool-band pairs first (they gate Q), then rest
        pair_order, slot, psize = [], {}, 0
        for opset in (hi_ops, lo_ops):
            for (a, b, o, d, c) in opset:
                key = (a, b, d)
                if key not in slot:
                    slot[key] = psize
                    pair_order.append(key)
                    psize += d
        prod_instrs = _merge_products(pair_order, slot)

        n_lo = max(2, min(3, B // 128)) if B else 0
        n_hi = max(2, min(4, (out_dim - B) // 128))
        dve_chunks = []
        for (clo, chi) in (_chunk_bounds(lo_ops, 0, B, bounds, n_lo) if lo_ops
                           else []):
            ops_c = [t for t in lo_ops if clo <= t[2] and t[2] + t[3] <= chi]
            dve_chunks.append(dict(lo=clo, hi=chi,
                                   acc_instrs=_plan_dve_chunk(ops_c, clo, chi,
                                                              slot)))
        pool_chunks = []
        qsize = 0
        if hi_ops:
            for (clo, chi) in _chunk_bounds(hi_ops, B, out_dim, bounds, n_hi):
                ops_c = [t for t in hi_ops if clo <= t[2] and t[2] + t[3] <= chi]
                qi, ai, qsize = _plan_pool_chunk(ops_c, clo, chi, slot, qsize)
                pool_chunks.append(dict(lo=clo, hi=chi, q_instrs=qi,
                                        add_instrs=ai, q_eng='DVE'))

        # costs
        rD, rP, rA = _RATES['DVE'], _RATES['Pool'], _RATES['ACT']
        cost_d = sum(T * pi['n'] * pi['d'] * rD['TT'] + rD['ovh']
                     for pi in prod_instrs)
        for ch in dve_chunks:
            cost_d += sum(T * q['n'] * q['d'] * rD[q['kind']] + rD['ovh']
                          for q in ch['acc_instrs'])
        cost_p = T * (out_dim - B) * rP['MEMSET'] + rP['ovh']
        for ch in pool_chunks:
            cost_p += sum(T * q['n'] * q['d'] * rP['ADD'] + rP['ovh']
                          for q in ch['add_instrs'])
        # assign Q chunks: ACT budget = loads+stores ~8000ns/group headroom;
        # greedily move whole chunks' Q to ACT while it helps the max
        cost_a = 8000.0
        qcosts = []
        for ch in pool_chunks:
            cd = sum(T * q['n'] * q['d'] * rD['TS'] + rD['ovh']
                     for q in ch['q_instrs'])
            ca = sum(T * q['n'] * q['d'] * rA['TS'] + rA['ovh']
                     for q in ch['q_instrs'])
            qcosts.append((cd, ca))
            cost_d += cd
        for i, ch in enumerate(pool_chunks):
            cd, ca = qcosts[i]
            if max(cost_d - cd, cost_a + ca, cost_p) < max(cost_d, cost_a, cost_p):
                ch['q_eng'] = 'ACT'
                cost_d -= cd
                cost_a += ca
        m = max(cost_d, cost_p, cost_a)
        return (m, B, prod_instrs, psize, qsize, dve_chunks, pool_chunks)

    best = None
    if force is not None:
        best = build_cfg(force)
    else:
        for B in bounds:
            if B < 128 or out_dim - B < 128:
                continue
            r = build_cfg(B)
            if best is None or r[0] < best[0]:
                best = r
    m, B, prod_instrs, psize, qsize, dve_chunks, pool_chunks = best
    return dict(T=T, B=B, prod_instrs=prod_instrs, psize=psize, qsize=qsize,
                dve_chunks=dve_chunks, pool_chunks=pool_chunks, makespan_est=m)


# ----------------------------------------------------------------------------
# Bass program
# ----------------------------------------------------------------------------

def _build_bass(plan, rows_per_core, rep_dim, out_dim, repeat=1, compute_repeat=1):
    import concourse.bass as bass
    import concourse.mybir as mybir
    from concourse.ap import AP
    from concourse.tile import TileContext
    import concourse.tile as _tile_mod
    from concourse.vector_clock import ScopedClock as _ScopedClock

    if not getattr(_tile_mod.TileContext, '_cg_drain_patched', False):
        def _split_drain_and_barrier(self, tick_clock, wait_clock):
            gc = tick_clock.global_clock
            VC = type(gc)
            procs = []
            for p in range(27):
                t = gc.peek_next(p) - 1
                if t > 0:
                    procs.append((p, t))
            for i in range(0, len(procs), 1):
                pc = VC()
                for p, t in procs[i:i + 1]:
                    for _ in range(t):
                        pc.advance(p)
                d = self.nc.sync.drain()
                wait_clock.add_sem_waits(d.ins, _ScopedClock({None: pc}))
            self.nc.all_engine_barrier()
            popped = self.nc._tile_sem_poison_stack.pop()
            assert popped is self._sem_poison
            self.nc.clear_and_free_semaphores(list(self.sems.allocated().values()))
            self.nc.all_engine_barrier()

        _tile_mod.TileContext._drain_and_barrier = _split_drain_and_barrier
        _tile_mod.TileContext._cg_drain_patched = True

    f32 = mybir.dt.float32
    f16 = mybir.dt.float16
    T = plan['T']
    n_groups = rows_per_core // (P_DIM * T)
    assert rows_per_core == n_groups * P_DIM * T

    nc = bass.Bass("TRN2")
    x1d = nc.declare_dram_parameter("x1", [rows_per_core, rep_dim], f16, isOutput=False)
    x2d = nc.declare_dram_parameter("x2", [rows_per_core, rep_dim], f16, isOutput=False)
    outd = nc.declare_dram_parameter("out", [rows_per_core, out_dim], f32, isOutput=True)

    def ap_custom(tile, base, dims):
        a = tile[:]
        aplist = [list(a.ap[0])] + [[s, n] for (s, n) in dims]
        return AP(a.tensor, a.offset + base, aplist)

    def dram_group_ap(dram, g, width, lo=0, hi=None):
        hi = width if hi is None else hi
        a = dram[:]
        return AP(a.tensor, g * T * P_DIM * width + lo,
                  [[width, P_DIM], [P_DIM * width, T], [1, hi - lo]])

    csz = plan['psize']
    qsz = plan['qsize']
    B = plan['B']

    with TileContext(nc) as tc:
        with (
            tc.tile_pool(name="io", bufs=2) as iop,
            tc.tile_pool(name="oo", bufs=2) as oop,
            tc.tile_pool(name="pp", bufs=2) as ppp,
            tc.tile_pool(name="qq", bufs=1) as qqp,
        ):
            for g in range(n_groups * repeat):
                g = g % n_groups
                X1 = iop.tile([P_DIM, T * rep_dim], f16, tag="X1")
                X2 = iop.tile([P_DIM, T * rep_dim], f16, tag="X2")
                O = oop.tile([P_DIM, T * out_dim], f32, tag="O")
                nc.sync.dma_start(X1[:], dram_group_ap(x1d, g, rep_dim))
                nc.scalar.dma_start(X2[:], dram_group_ap(x2d, g, rep_dim))
                SCR = iop.tile([P_DIM, 64], f16, tag="SCR")
                SCRP = iop.tile([P_DIM, 160], f16, tag="SCRP")
                SCA = iop.tile([P_DIM, 64], f32, tag="SCA")
                # DVE absorbers for the two loads (TT products can't wait)
                nc.vector.tensor_copy(SCR[:, 0:2], X1[:, 0:2])
                nc.vector.tensor_copy(SCR[:, 2:4], X2[:, 0:2])

                for _rep in range(compute_repeat):
                    P = ppp.tile([P_DIM, T * csz], f16, tag="P")
                    Q = qqp.tile([P_DIM, T * qsz], f16, tag="Q") if qsz else None
                    for pi in plan['prod_instrs']:
                        dims = [(csz, T), (pi['ds'], pi['n']), (1, pi['d'])]
                        nc.vector.tensor_tensor(
                            ap_custom(P, pi['pslot'], dims),
                            ap_custom(X1, pi['a'],
                                      [(rep_dim, T), (pi['da'], pi['n']), (1, pi['d'])]),
                            ap_custom(X2, pi['b'],
                                      [(rep_dim, T), (pi['db'], pi['n']), (1, pi['d'])]),
                            mybir.AluOpType.mult,
                        )
                    last = _rep == compute_repeat - 1
                    scr_i = [0]
                    sca_i = [0]

                    def act_store(lo, hi):
                        # strided ACT absorber over the chunk, then ACT store
                        nblk = (hi - lo) // 32
                        nc.scalar.activation(
                            ap_custom(SCA, sca_i[0], [(1, 1), (1, nblk)]),
                            ap_custom(O, lo, [(out_dim, 1), (32, nblk)]),
                            mybir.ActivationFunctionType.Copy)
                        sca_i[0] += nblk
                        nc.scalar.dma_start(
                            dram_group_ap(outd, g, out_dim, lo, hi),
                            ap_custom(O, lo, [(out_dim, T), (1, hi - lo)]))

                    # Pool band: memset, then per chunk Q -> absorber -> adds
                    if plan['pool_chunks']:
                        nc.gpsimd.memset(
                            ap_custom(O, B, [(out_dim, T), (1, out_dim - B)]),
                            0.0)
                    for ch in plan['pool_chunks']:
                        qeng = nc.vector if ch['q_eng'] == 'DVE' else nc.scalar
                        qlo = min(q['o'] for q in ch['q_instrs'])
                        qhi = max(q['o'] + q['n'] * q['d'] for q in ch['q_instrs'])
                        for q in ch['q_instrs']:
                            w = q['n'] * q['d']
                            q_ap = ap_custom(Q, q['o'], [(qsz, T), (1, w)])
                            p_ap = ap_custom(P, q['pslot'], [(csz, T), (1, w)])
                            if ch['q_eng'] == 'DVE':
                                nc.vector.tensor_scalar_mul(q_ap, p_ap,
                                                            float(q['c']))
                            else:
                                nc.scalar.activation(
                                    q_ap, p_ap,
                                    mybir.ActivationFunctionType.Copy,
                                    scale=float(q['c']))
                        # Pool absorber: strided copy touching every 32-block
                        # of this chunk's Q region (1 producer-engine wait)
                        nblk = max(1, (qhi - qlo + 31) // 32)
                        cell = scr_i[0]
                        scr_i[0] = cell + nblk
                        nc.gpsimd.tensor_copy(
                            ap_custom(SCRP, cell, [(1, 1), (1, nblk)]),
                            ap_custom(Q, qlo, [(1, 1), (32, nblk)]))
                        for q in ch['add_instrs']:
                            w = q['n'] * q['d']
                            o_ap = ap_custom(O, q['o'], [(out_dim, T), (1, w)])
                            q_ap = ap_custom(Q, q['pslot'], [(qsz, T), (1, w)])
                            nc.gpsimd.tensor_tensor(o_ap, o_ap, q_ap,
                                                    mybir.AluOpType.add)
                        if last:
                            act_store(ch['lo'], ch['hi'])

                    # DVE band: TS/STT chunks
                    for ch in plan['dve_chunks']:
                        for qi in ch['acc_instrs']:
                            w = qi['n'] * qi['d']
                            o_ap = ap_custom(O, qi['o'], [(out_dim, T), (1, w)])
                            p_ap = ap_custom(P, qi['pslot'], [(csz, T), (1, w)])
                            if qi['kind'] == 'TS':
                                nc.vector.tensor_scalar_mul(o_ap, p_ap,
                                                            float(qi['c']))
                            else:
                                nc.vector.scalar_tensor_tensor(
                                    out=o_ap, in0=p_ap, scalar=float(qi['c']),
                                    in1=o_ap,
                                    op0=mybir.AluOpType.mult,
                                    op1=mybir.AluOpType.add,
                                )
                        if last:
                            act_store(ch['lo'], ch['hi'])
    return nc


# ----------------------------------------------------------------------------
# Entry point
# ----------------------------------------------------------------------------

def kernel(x1, x2, cg_tilde, repids_in1, repids_in2, repids_out, out_dim):
    from concourse.bass_utils import run_bass_kernel_spmd

    x1 = np.asarray(x1, dtype=np.float32).astype(np.float16)
    x2 = np.asarray(x2, dtype=np.float32).astype(np.float16)
    cg = np.asarray(cg_tilde, dtype=np.float32)
    r1 = np.asarray(repids_in1).astype(np.int64)
    r2 = np.asarray(repids_in2).astype(np.int64)
    ro = np.asarray(repids_out).astype(np.int64)
    out_dim = int(out_dim)

    n, rep_dim = x1.shape
    rows_per_core = n // N_CORES

    key = (rows_per_core, rep_dim, out_dim, cg.tobytes(), r1.tobytes(),
           r2.tobytes(), ro.tobytes())
    cache_key = hash(key)
    if cache_key not in _BUILD_CACHE:
        # B = out_dim: all accumulation on DVE (TS/STT are DVE-only opcodes
        # on real TRN2, and cross-engine waits overflow the 1-wait ISA limit)
        plan = _build_plan(cg, r1, r2, ro, out_dim, force=out_dim)
        nc = _build_bass(plan, rows_per_core, rep_dim, out_dim)
        _BUILD_CACHE[cache_key] = nc
    nc = _BUILD_CACHE[cache_key]

    in_maps = [
        {"x1": x1[i*rows_per_core:(i+1)*rows_per_core],
         "x2": x2[i*rows_per_core:(i+1)*rows_per_core]}
        for i in range(N_CORES)
    ]
    res = run_bass_kernel_spmd(nc, in_maps, list(range(N_CORES)))
    out = np.concatenate([res.results[i]["out"] for i in range(N_CORES)], axis=0)
    return out


# revision 22
# speedup vs baseline: 1.0382x; 1.0382x over previous
"""CGCoupler Trainium2 Bass kernel: fp16 inputs, legal DVE/Pool/ACT split.

out[n, ro[k]] += x1[n, r1[k]] * x2[n, r2[k]] * cg[k]  for all k, rows n.

The CG index tables decompose into 147 contiguous-slice FMAs per row
(out[:, o:o+d] += c * x1[:, a:a+d] * x2[:, b:b+d], d in {32,64}).

Engine legality on real TRN2 (walrus ISA checks; CoreSim is looser):
TensorScalar/ScalarTensorTensor run ONLY on DVE; the Pool engine supports
TensorTensor/memset/copy/DMA; ACT supports activation (copy with scale).
ISA structs carry at most ONE embedded sync wait, and 4D-AP TensorTensor
carries none, so every cross-engine handoff goes through a wait-capable
absorber instruction on the consuming engine.

Structure:
 - x1/x2 are fp16 (converted on host).  X1 loads issue from SP, X2 from ACT
   (parallel HWDGE queues).  All products (fp16 TT, 0.52 ns/elem) run on DVE
   into a shared P buffer.
 - The output space splits at boundary B.  The DVE band accumulates with
   TS/STT as usual.  The Pool band instead: DVE/ACT compute per-term scaled
   products Q = c*P (DVE tensor_scalar at 0.26 ns/elem fp16, ACT activation
   copy-with-scale at 0.833; whole chunks assigned to one producer), Pool
   memsets its O band and accumulates with 3D TT-adds (0.833 ns/elem,
   coefficient-free so runs merge across ranges by layer).  A strided Pool
   copy per chunk absorbs the producer wait (robust to scheduler
   reordering); each chunk's store issues from ACT after a strided ACT
   absorber, so stores carry only their DMA-ring wait.
 - Rows ride the 128 SBUF partitions, T=8 row-tiles fold per instruction,
   2 groups per core; O is single-buffered (fits SBUF), X/P double-buffered.

Data-parallel across 8 NeuronCores: each core processes 2048 rows.
"""
import numpy as np

N_CORES = 8
P_DIM = 128
T_FOLD = 8

_BUILD_CACHE = {}


# ----------------------------------------------------------------------------
# Planning
# ----------------------------------------------------------------------------

def _extract_sliceops(cg, r1, r2, ro):
    M = len(cg)
    ops = []
    k = 0
    while k < M:
        j = k + 1
        while (j < M and r1[j] == r1[j-1] + 1 and r2[j] == r2[j-1] + 1
               and ro[j] == ro[j-1] + 1 and cg[j] == cg[k]):
            j += 1
        ops.append((int(r1[k]), int(r2[k]), int(ro[k]), j - k, float(cg[k])))
        k = j
    return ops


_RATES = {
    'DVE':  dict(TT=0.5208, STT=1.0417, TS=0.2604, ovh=105.0),
    'Pool': dict(ADD=0.8333, MEMSET=0.8333, ovh=40.0),
    'ACT':  dict(TS=0.8333, ovh=60.0),
}


def _merge_products(pairs, slot):
    instrs = []
    i = 0
    while i < len(pairs):
        a0, b0, d0 = pairs[i]
        s0 = slot[pairs[i]]
        j = i + 1
        da = db = ds = None
        while j < len(pairs):
            a1, b1, d1 = pairs[j]
            if d1 != d0:
                break
            nda = a1 - pairs[j-1][0]
            ndb = b1 - pairs[j-1][1]
            nds = slot[pairs[j]] - slot[pairs[j-1]]
            if da is None:
                da, db, ds = nda, ndb, nds
            elif (nda, ndb, nds) != (da, db, ds):
                break
            j += 1
        n = j - i
        if n == 1:
            da = db = ds = 0
        instrs.append(dict(pslot=s0, a=a0, b=b0, d=d0, da=da, db=db, ds=ds, n=n))
        i = j
    return instrs


def _merge_runs(qs, need_c):
    """Merge ops with contiguous (o, pslot) runs of equal d (and equal c when
    need_c).  qs must carry o/pslot/c/d/kind."""
    instrs = []
    i = 0
    while i < len(qs):
        q0 = qs[i]
        j = i + 1
        while j < len(qs):
            q1, qp = qs[j], qs[j-1]
            if q1['kind'] != q0['kind'] or q1['d'] != q0['d']:
                break
            if need_c and q1['c'] != q0['c']:
                break
            if q1['o'] - qp['o'] != q0['d'] or q1['pslot'] - qp['pslot'] != q0['d']:
                break
            j += 1
        n = j - i
        instrs.append(dict(kind=q0['kind'], o=q0['o'], pslot=q0['pslot'],
                           c=q0['c'], d=q0['d'], n=n))
        i = j
    return instrs


def _plan_dve_chunk(ops_c, lo, hi, slot):
    covered = np.zeros(hi - lo, bool)
    qs = []
    raw = sorted(ops_c, key=lambda t: (-t[3], t[4], t[2]))
    for (a, b, o, d, c) in raw:
        rng = slice(o - lo, o - lo + d)
        kind = 'TS' if not covered[rng].any() else 'STT'
        assert kind == 'TS' or covered[rng].all()
        covered[rng] = True
        qs.append(dict(o=o, pslot=slot[(a, b, d)], c=c, d=d, kind=kind))
    assert covered.all()
    qs = sorted(qs, key=lambda q: (q['kind'] != 'TS', -q['d'], q['c'],
                                   q['o'], q['pslot']))
    return _merge_runs(qs, need_c=True)


def _plan_pool_chunk(ops_c, lo, hi, slot, qbase):
    """Pool-band chunk: per-op Q slots laid out layer-major so TT-adds merge
    across ranges without a coefficient constraint."""
    layer_of = {}
    items = []
    for (a, b, o, d, c) in sorted(ops_c, key=lambda t: (t[2], -t[3])):
        l = layer_of.get(o, 0)
        layer_of[o] = l + 1
        items.append(dict(o=o, pslot=slot[(a, b, d)], c=c, d=d, layer=l))
    items.sort(key=lambda q: (q['layer'], q['o']))
    qoff = qbase
    for it in items:
        it['qslot'] = qoff
        qoff += it['d']
    # Q instructions: qslot contiguous by construction; merge needs same c
    qgen = [dict(kind='Q', o=it['qslot'], pslot=it['pslot'], c=it['c'],
                 d=it['d']) for it in items]
    q_instrs = _merge_runs(qgen, need_c=True)
    # TT-adds: out[o] += Q[qslot]; merge runs regardless of c
    agen = [dict(kind='ADD', o=it['o'], pslot=it['qslot'], c=0.0, d=it['d'])
            for it in items]
    add_instrs = _merge_runs(agen, need_c=False)
    return q_instrs, add_instrs, qoff


def _chunk_bounds(band_ops, out_lo, out_hi, bounds, n_chunks, taper=0.8):
    valid = [b for b in bounds if out_lo < b < out_hi]
    work = np.zeros(out_hi - out_lo)
    for (a, b, o, d, c) in band_ops:
        work[o - out_lo:o - out_lo + d] += 1
    total = work.sum()
    weights = np.array([taper ** i for i in range(n_chunks)])
    weights = weights / weights.sum()
    chunks = []
    cur_lo = out_lo
    remaining = n_chunks
    for ci in range(n_chunks - 1):
        target = total * weights[ci]
        best_b, best_gap = None, None
        for b in valid:
            if b - cur_lo < 128 or out_hi - b < 128 * (remaining - 1):
                continue
            w = work[cur_lo - out_lo:b - out_lo].sum()
            gap = abs(w - target)
            if best_gap is None or gap < best_gap:
                best_b, best_gap = b, gap
        if best_b is None:
            break
        chunks.append((cur_lo, best_b))
        cur_lo = best_b
        remaining -= 1
    chunks.append((cur_lo, out_hi))
    return chunks


def _build_plan(cg, r1, r2, ro, out_dim, T=T_FOLD, force=None):
    ops = _extract_sliceops(cg, r1, r2, ro)

    cand = set(range(32, out_dim, 32))
    for (_, _, o, d, _) in ops:
        for x in range(o + 32, o + d, 32):
            cand.discard(x)
    bounds = sorted(cand)

    def build_cfg(B):
        # DVE band = [0, B), Pool band = [B, out_dim)
        lo_ops = [t for t in ops if t[2] + t[3] <= B]
        hi_ops = [t for t in ops if t[2] >= B]
        assert len(lo_ops) + len(hi_ops) == len(ops)
        # shared P layout: pool-band pairs first (they gate Q), then rest
        pair_order, slot, psize = [], {}, 0
        for opset in (hi_ops, lo_ops):
            for (a, b, o, d, c) in opset:
                key = (a, b, d)
                if key not in slot:
                    slot[key] = psize
                    pair_order.append(key)
                    psize += d
        prod_instrs = _merge_products(pair_order, slot)

        n_lo = max(2, min(3, B // 128)) if B else 0
        n_hi = max(2, min(4, (out_dim - B) // 128))
        dve_chunks = []
        for (clo, chi) in (_chunk_bounds(lo_ops, 0, B, bounds, n_lo) if lo_ops
                           else []):
            ops_c = [t for t in lo_ops if clo <= t[2] and t[2] + t[3] <= chi]
            dve_chunks.append(dict(lo=clo, hi=chi,
                                   acc_instrs=_plan_dve_chunk(ops_c, clo, chi,
                                                              slot)))
        pool_chunks = []
        qsize = 0
        if hi_ops:
            for (clo, chi) in _chunk_bounds(hi_ops, B, out_dim, bounds, n_hi):
                ops_c = [t for t in hi_ops if clo <= t[2] and t[2] + t[3] <= chi]
                qi, ai, qsize = _plan_pool_chunk(ops_c, clo, chi, slot, qsize)
                pool_chunks.append(dict(lo=clo, hi=chi, q_instrs=qi,
                                        add_instrs=ai, q_eng='DVE'))

        # costs
        rD, rP, rA = _RATES['DVE'], _RATES['Pool'], _RATES['ACT']
        cost_d = sum(T * pi['n'] * pi['d'] * rD['TT'] + rD['ovh']
                     for pi in prod_instrs)
        for ch in dve_chunks:
            cost_d += sum(T * q['n'] * q['d'] * rD[q['kind']] + rD['ovh']
                          for q in ch['acc_instrs'])
        cost_p = T * (out_dim - B) * rP['MEMSET'] + rP['ovh']
        for ch in pool_chunks:
            cost_p += sum(T * q['n'] * q['d'] * rP['ADD'] + rP['ovh']
                          for q in ch['add_instrs'])
        # assign Q chunks: ACT budget = loads+stores ~8000ns/group headroom;
        # greedily move whole chunks' Q to ACT while it helps the max
        cost_a = 8000.0
        qcosts = []
        for ch in pool_chunks:
            cd = sum(T * q['n'] * q['d'] * rD['TS'] + rD['ovh']
                     for q in ch['q_instrs'])
            ca = sum(T * q['n'] * q['d'] * rA['TS'] + rA['ovh']
                     for q in ch['q_instrs'])
            qcosts.append((cd, ca))
            cost_d += cd
        for i, ch in enumerate(pool_chunks):
            cd, ca = qcosts[i]
            if max(cost_d - cd, cost_a + ca, cost_p) < max(cost_d, cost_a, cost_p):
                ch['q_eng'] = 'ACT'
                cost_d -= cd
                cost_a += ca
        m = max(cost_d, cost_p, cost_a)
        return (m, B, prod_instrs, psize, qsize, dve_chunks, pool_chunks)

    best = None
    if force is not None:
        best = build_cfg(force)
    else:
        for B in bounds:
            if B < 128 or out_dim - B < 128:
                continue
            r = build_cfg(B)
            if best is None or r[0] < best[0]:
                best = r
    m, B, prod_instrs, psize, qsize, dve_chunks, pool_chunks = best
    return dict(T=T, B=B, prod_instrs=prod_instrs, psize=psize, qsize=qsize,
                dve_chunks=dve_chunks, pool_chunks=pool_chunks, makespan_est=m)


# ----------------------------------------------------------------------------
# Bass program
# ----------------------------------------------------------------------------

def _build_bass(plan, rows_per_core, rep_dim, out_dim, repeat=1, compute_repeat=1):
    import concourse.bass as bass
    import concourse.mybir as mybir
    from concourse.ap import AP
    from concourse.tile import TileContext
    import concourse.tile as _tile_mod
    from concourse.vector_clock import ScopedClock as _ScopedClock

    if not getattr(_tile_mod.TileContext, '_cg_drain_patched', False):
        def _split_drain_and_barrier(self, tick_clock, wait_clock):
            gc = tick_clock.global_clock
            VC = type(gc)
            procs = []
            for p in range(27):
                t = gc.peek_next(p) - 1
                if t > 0:
                    procs.append((p, t))
            for i in range(0, len(procs), 1):
                pc = VC()
                for p, t in procs[i:i + 1]:
                    for _ in range(t):
                        pc.advance(p)
                d = self.nc.sync.drain()
                wait_clock.add_sem_waits(d.ins, _ScopedClock({None: pc}))
            self.nc.all_engine_barrier()
            popped = self.nc._tile_sem_poison_stack.pop()
            assert popped is self._sem_poison
            self.nc.clear_and_free_semaphores(list(self.sems.allocated().values()))
            self.nc.all_engine_barrier()

        _tile_mod.TileContext._drain_and_barrier = _split_drain_and_barrier
        _tile_mod.TileContext._cg_drain_patched = True

    f32 = mybir.dt.float32
    f16 = mybir.dt.float16
    T = plan['T']
    n_groups = rows_per_core // (P_DIM * T)
    assert rows_per_core == n_groups * P_DIM * T

    nc = bass.Bass("TRN2")
    x1d = nc.declare_dram_parameter("x1", [rows_per_core, rep_dim], f16, isOutput=False)
    x2d = nc.declare_dram_parameter("x2", [rows_per_core, rep_dim], f16, isOutput=False)
    outd = nc.declare_dram_parameter("out", [rows_per_core, out_dim], f32, isOutput=True)

    def ap_custom(tile, base, dims):
        a = tile[:]
        aplist = [list(a.ap[0])] + [[s, n] for (s, n) in dims]
        return AP(a.tensor, a.offset + base, aplist)

    def dram_group_ap(dram, g, width, lo=0, hi=None):
        hi = width if hi is None else hi
        a = dram[:]
        return AP(a.tensor, g * T * P_DIM * width + lo,
                  [[width, P_DIM], [P_DIM * width, T], [1, hi - lo]])

    csz = plan['psize']
    qsz = plan['qsize']
    B = plan['B']

    nbuf = 2 if n_groups > 1 else 1
    with TileContext(nc) as tc:
        with (
            tc.tile_pool(name="io", bufs=nbuf) as iop,
            tc.tile_pool(name="oo", bufs=nbuf) as oop,
            tc.tile_pool(name="pp", bufs=nbuf) as ppp,
            tc.tile_pool(name="qq", bufs=1) as qqp,
        ):
            for g in range(n_groups * repeat):
                g = g % n_groups
                X1 = iop.tile([P_DIM, T * rep_dim], f16, tag="X1")
                X2 = iop.tile([P_DIM, T * rep_dim], f16, tag="X2")
                O = oop.tile([P_DIM, T * out_dim], f32, tag="O")
                nc.sync.dma_start(X1[:], dram_group_ap(x1d, g, rep_dim))
                nc.scalar.dma_start(X2[:], dram_group_ap(x2d, g, rep_dim))
                SCR = iop.tile([P_DIM, 64], f16, tag="SCR")
                SCRP = iop.tile([P_DIM, 160], f16, tag="SCRP")
                SCA = iop.tile([P_DIM, 64], f32, tag="SCA")
                # DVE absorbers for the two loads (TT products can't wait)
                nc.vector.tensor_copy(SCR[:, 0:2], X1[:, 0:2])
                nc.vector.tensor_copy(SCR[:, 2:4], X2[:, 0:2])

                for _rep in range(compute_repeat):
                    P = ppp.tile([P_DIM, T * csz], f16, tag="P")
                    Q = qqp.tile([P_DIM, T * qsz], f16, tag="Q") if qsz else None
                    for pi in plan['prod_instrs']:
                        dims = [(csz, T), (pi['ds'], pi['n']), (1, pi['d'])]
                        nc.vector.tensor_tensor(
                            ap_custom(P, pi['pslot'], dims),
                            ap_custom(X1, pi['a'],
                                      [(rep_dim, T), (pi['da'], pi['n']), (1, pi['d'])]),
                            ap_custom(X2, pi['b'],
                                      [(rep_dim, T), (pi['db'], pi['n']), (1, pi['d'])]),
                            mybir.AluOpType.mult,
                        )
                    last = _rep == compute_repeat - 1
                    scr_i = [0]
                    sca_i = [0]

                    def act_store(lo, hi):
                        # strided ACT absorber over the chunk, then ACT store
                        nblk = (hi - lo) // 32
                        nc.scalar.activation(
                            ap_custom(SCA, sca_i[0], [(1, 1), (1, nblk)]),
                            ap_custom(O, lo, [(out_dim, 1), (32, nblk)]),
                            mybir.ActivationFunctionType.Copy)
                        sca_i[0] += nblk
                        nc.scalar.dma_start(
                            dram_group_ap(outd, g, out_dim, lo, hi),
                            ap_custom(O, lo, [(out_dim, T), (1, hi - lo)]))

                    # Pool band: memset, then per chunk Q -> absorber -> adds
                    if plan['pool_chunks']:
                        nc.gpsimd.memset(
                            ap_custom(O, B, [(out_dim, T), (1, out_dim - B)]),
                            0.0)
                    for ch in plan['pool_chunks']:
                        qeng = nc.vector if ch['q_eng'] == 'DVE' else nc.scalar
                        qlo = min(q['o'] for q in ch['q_instrs'])
                        qhi = max(q['o'] + q['n'] * q['d'] for q in ch['q_instrs'])
                        for q in ch['q_instrs']:
                            w = q['n'] * q['d']
                            q_ap = ap_custom(Q, q['o'], [(qsz, T), (1, w)])
                            p_ap = ap_custom(P, q['pslot'], [(csz, T), (1, w)])
                            if ch['q_eng'] == 'DVE':
                                nc.vector.tensor_scalar_mul(q_ap, p_ap,
                                                            float(q['c']))
                            else:
                                nc.scalar.activation(
                                    q_ap, p_ap,
                                    mybir.ActivationFunctionType.Copy,
                                    scale=float(q['c']))
                        # Pool absorber: strided copy touching every 32-block
                        # of this chunk's Q region (1 producer-engine wait)
                        nblk = max(1, (qhi - qlo + 31) // 32)
                        cell = scr_i[0]
                        scr_i[0] = cell + nblk
                        nc.gpsimd.tensor_copy(
                            ap_custom(SCRP, cell, [(1, 1), (1, nblk)]),
                            ap_custom(Q, qlo, [(1, 1), (32, nblk)]))
                        for q in ch['add_instrs']:
                            w = q['n'] * q['d']
                            o_ap = ap_custom(O, q['o'], [(out_dim, T), (1, w)])
                            q_ap = ap_custom(Q, q['pslot'], [(qsz, T), (1, w)])
                            nc.gpsimd.tensor_tensor(o_ap, o_ap, q_ap,
                                                    mybir.AluOpType.add)
                        if last:
                            act_store(ch['lo'], ch['hi'])

                    # DVE band: TS/STT chunks
                    for ch in plan['dve_chunks']:
                        for qi in ch['acc_instrs']:
                            w = qi['n'] * qi['d']
                            o_ap = ap_custom(O, qi['o'], [(out_dim, T), (1, w)])
                            p_ap = ap_custom(P, qi['pslot'], [(csz, T), (1, w)])
                            if qi['kind'] == 'TS':
                                nc.vector.tensor_scalar_mul(o_ap, p_ap,
                                                            float(qi['c']))
                            else:
                                nc.vector.scalar_tensor_tensor(
                                    out=o_ap, in0=p_ap, scalar=float(qi['c']),
                                    in1=o_ap,
                                    op0=mybir.AluOpType.mult,
                                    op1=mybir.AluOpType.add,
                                )
                        if last:
                            act_store(ch['lo'], ch['hi'])
    return nc


# ----------------------------------------------------------------------------
# Entry point
# ----------------------------------------------------------------------------

def kernel(x1, x2, cg_tilde, repids_in1, repids_in2, repids_out, out_dim):
    from concourse.bass_utils import run_bass_kernel_spmd

    x1 = np.asarray(x1, dtype=np.float32).astype(np.float16)
    x2 = np.asarray(x2, dtype=np.float32).astype(np.float16)
    cg = np.asarray(cg_tilde, dtype=np.float32)
    r1 = np.asarray(repids_in1).astype(np.int64)
    r2 = np.asarray(repids_in2).astype(np.int64)
    ro = np.asarray(repids_out).astype(np.int64)
    out_dim = int(out_dim)

    n, rep_dim = x1.shape
    rows_per_core = n // N_CORES

    key = (rows_per_core, rep_dim, out_dim, cg.tobytes(), r1.tobytes(),
           r2.tobytes(), ro.tobytes())
    cache_key = hash(key)
    if cache_key not in _BUILD_CACHE:
        # B = out_dim: all accumulation on DVE (TS/STT are DVE-only opcodes
        # on real TRN2, and cross-engine waits overflow the 1-wait ISA limit)
        T = 16 if rows_per_core % (P_DIM * 16) == 0 else T_FOLD
        plan = _build_plan(cg, r1, r2, ro, out_dim, T=T, force=out_dim)
        nc = _build_bass(plan, rows_per_core, rep_dim, out_dim)
        _BUILD_CACHE[cache_key] = nc
    nc = _BUILD_CACHE[cache_key]

    in_maps = [
        {"x1": x1[i*rows_per_core:(i+1)*rows_per_core],
         "x2": x2[i*rows_per_core:(i+1)*rows_per_core]}
        for i in range(N_CORES)
    ]
    res = run_bass_kernel_spmd(nc, in_maps, list(range(N_CORES)))
    out = np.concatenate([res.results[i]["out"] for i in range(N_CORES)], axis=0)
    return out
